# revision 5
# baseline (speedup 1.0000x reference)
"""DeepNorm encoder layer on 8 TRN2 NeuronCores (Bass/Tile, SPMD).

Sharding: batch (2) x row-chunks (4) data-parallel for queries/FF; K/V
projections tensor-parallel within each 4-core batch group (each core
projects 4 heads' K/V for the full sequence, AllGathered to all cores of
the group). Attention computes all 16 heads for the core's 512 query rows.

Matmuls run in bf16 with fp32 PSUM accumulation; residuals/LN in fp32.
Activations feeding matmul RHS are kept transposed ([feature, token]);
LN/residual rows are [token, feature]. Head pairs share one 2-bank PSUM
scores tile (row-packed K=64 matmuls) drained by a single fused ACT exp.
"""
import os
import contextlib
import numpy as np
import ml_dtypes

import concourse.bass as bass
import concourse.mybir as mybir
import concourse.tile as tile
from concourse import bacc
from concourse.bass_utils import run_bass_kernel_spmd
from concourse.masks import make_identity

P = 128
T = 2048          # sequence length
C = 1024          # d_model
H = 16            # heads
DH = 64           # head dim
FF = 4096         # d_ff
NCORES = 8
R = 512           # query rows per core
HL = H // 4       # heads projected locally per core (4)
CL = HL * DH      # local K/V width (256)
ALPHA = 2.0
EPS = 1e-5

BF = mybir.dt.bfloat16
F32 = mybir.dt.float32
AF = mybir.ActivationFunctionType
ALU = mybir.AluOpType

CK = C // P    # 8 chunks of d_model
FK = FF // P   # 32 chunks of d_ff
TK = T // P    # 16 chunks of sequence
NT = T // 512  # 4 free-dim tiles of sequence
RM = R // P    # 4 row chunks per core
GROUPS = [[0, 1, 2, 3], [4, 5, 6, 7]]

_CACHE = {}


def _build(trivial):
    """trivial=True: g1=g2=1, be1=be2=b1=b2=0 (the reference's actual
    parameters) -> skip the gain/bias application passes."""
    nc = bacc.Bacc("TRN2", target_bir_lowering=False, debug=False,
                   num_devices=NCORES)

    xT_d = nc.dram_tensor("xT", [C, T], BF, kind="ExternalInput").ap()
    xq_d = nc.dram_tensor("xTq", [C, R], BF, kind="ExternalInput").ap()
    xr_d = nc.dram_tensor("x_rows", [R, C], F32, kind="ExternalInput").ap()
    wq_d = nc.dram_tensor("wq", [C, C], BF, kind="ExternalInput").ap()
    wk_d = nc.dram_tensor("wk_sl", [C, CL], BF, kind="ExternalInput").ap()
    wv_d = nc.dram_tensor("wv_sl", [C, CL], BF, kind="ExternalInput").ap()
    wo_d = nc.dram_tensor("wo", [C, C], BF, kind="ExternalInput").ap()
    w1_d = nc.dram_tensor("w1", [C, FF], BF, kind="ExternalInput").ap()
    w2_d = nc.dram_tensor("w2", [FF, C], BF, kind="ExternalInput").ap()
    b1_d = nc.dram_tensor("b1t", [P, FK], F32, kind="ExternalInput").ap()
    b2_d = nc.dram_tensor("b2", [C], F32, kind="ExternalInput").ap()
    g1_d = nc.dram_tensor("g1", [C], F32, kind="ExternalInput").ap()
    be1_d = nc.dram_tensor("be1", [C], F32, kind="ExternalInput").ap()
    g2_d = nc.dram_tensor("g2", [C], F32, kind="ExternalInput").ap()
    be2_d = nc.dram_tensor("be2", [C], F32, kind="ExternalInput").ap()
    out_d = nc.dram_tensor("out", [R, C], F32, kind="ExternalOutput").ap()

    # DRAM bounce buffers for the K/V AllGather
    ktl_dram = nc.dram_tensor("ktl", [CL, T], BF).ap()
    vl_dram = nc.dram_tensor("vl", [T, CL], BF).ap()
    ktg_dram = nc.dram_tensor("ktg", [C, T], BF).ap()
    vg_dram = nc.dram_tensor("vg", [4, T, CL], BF).ap()

    def bcast(ap):  # [N] dram vector -> AP broadcasting to 128 partitions
        return bass.AP(tensor=ap.tensor, offset=ap.offset,
                       ap=[[0, P]] + list(ap.ap))

    with tile.TileContext(nc) as tc, contextlib.ExitStack() as es:
        const = es.enter_context(tc.tile_pool(name="const", bufs=1))
        zeros1 = const.tile([P, 1], F32); nc.vector.memset(zeros1, 0.0)
        ones1 = const.tile([P, 1], F32); nc.vector.memset(ones1, 1.0)
        eps1 = const.tile([P, 1], F32); nc.vector.memset(eps1, EPS)
        ones_f = const.tile([P, 512], F32); nc.vector.memset(ones_f, 1.0)
        ones_w = const.tile([P, DH], F32); nc.vector.memset(ones_w, 1.0)
        ident = const.tile([P, P], BF); make_identity(nc, ident)
        b1t = const.tile([P, FK], F32); nc.sync.dma_start(out=b1t, in_=b1_d)

        rows_pool = es.enter_context(tc.tile_pool(name="rows", bufs=1))
        x_rows = rows_pool.tile([P, RM, C], F32)
        nc.sync.dma_start(out=x_rows, in_=xr_d.rearrange("(m p) c -> p m c", p=P))
        ctxt = rows_pool.tile([P, CK, R], BF)   # ctx^T [1024, 512]

        # ============ Phases A+B: projections + attention ============
        with tc.tile_pool(name="attn", bufs=1) as attn:
            qt_sb = attn.tile([P, CK, R], BF)           # Q^T [1024, 512]
            kt_sb = attn.tile([P, CK, T], BF)           # K^T [1024, 2048] (all heads)
            vaug = attn.tile([P, TK, H, DH + 1], BF)    # V rows + ones col

            with (
                tc.tile_pool(name="pa", bufs=1) as pa,
                tc.tile_pool(name="pwstr", bufs=4) as pwstr,
                tc.tile_pool(name="psA", bufs=4, space="PSUM") as psA,
            ):
                xt_sb = pa.tile([P, CK, T], BF, tag="xt")
                xt_r = xT_d.rearrange("(k p) t -> p k t", p=P)
                for k in range(CK):
                    for hh in range(2):
                        nc.sync.dma_start(
                            out=xt_sb[:, k, hh * 1024:(hh + 1) * 1024],
                            in_=xt_r[:, k, hh * 1024:(hh + 1) * 1024])
                xq_sb = pa.tile([P, CK, R], BF, tag="xq")
                nc.sync.dma_start(out=xq_sb,
                                  in_=xq_d.rearrange("(k p) t -> p k t", p=P))
                nc.vector.memset(vaug[:, :, :, DH:DH + 1], 1.0)

                # local K^T slice [CL, T] (this core's 4 heads, full sequence)
                wk_r = wk_d.rearrange("(k p) m -> p k m", p=P)
                wksl = pwstr.tile([P, CK, CL], BF, tag="wksl")
                nc.sync.dma_start(out=wksl, in_=wk_r)
                ktl_sb = pa.tile([P, 2, T], BF, tag="ktl")
                for m in range(2):
                    for n in range(NT):
                        ps = psA.tile([P, 512], F32, tag="pa", name=f"psk{m}{n}")
                        for k in range(CK):
                            nc.tensor.matmul(ps, wksl[:, k, m * P:(m + 1) * P],
                                             xt_sb[:, k, n * 512:(n + 1) * 512],
                                             start=(k == 0), stop=(k == CK - 1))
                        nc.vector.tensor_copy(ktl_sb[:, m, n * 512:(n + 1) * 512],
                                              ps)
                nc.sync.dma_start(out=ktl_dram.rearrange("(m p) t -> p m t", p=P),
                                  in_=ktl_sb)
                # local V slice [T, CL]
                wv_r = wv_d.rearrange("(k p) m -> p k m", p=P)
                wvsl = pwstr.tile([P, CK, CL], BF, tag="wvsl")
                nc.sync.dma_start(out=wvsl, in_=wv_r)
                vl_sb = pa.tile([P, TK, CL], BF, tag="vl")
                for t in range(TK):
                    ps = psA.tile([P, 512], F32, tag="pa", name=f"psv{t}")
                    for k in range(CK):
                        nc.tensor.matmul(ps[:, 0:CL],
                                         xt_sb[:, k, t * P:(t + 1) * P],
                                         wvsl[:, k, :],
                                         start=(k == 0), stop=(k == CK - 1))
                    nc.vector.tensor_copy(vl_sb[:, t, :], ps[:, 0:CL])
                nc.sync.dma_start(out=vl_dram.rearrange("(t p) m -> p t m", p=P),
                                  in_=vl_sb)

                # AllGather K^T and V within 4-core batch groups
                nc.gpsimd.collective_compute(
                    "AllGather", ALU.bypass, replica_groups=GROUPS,
                    ins=[ktl_dram], outs=[ktg_dram])
                nc.gpsimd.collective_compute(
                    "AllGather", ALU.bypass, replica_groups=GROUPS,
                    ins=[vl_dram], outs=[vg_dram])
                nc.sync.dma_start(out=kt_sb,
                                  in_=ktg_dram.rearrange("(k p) t -> p k t", p=P))
                for r in range(4):
                    vg_r = vg_dram[r].rearrange("(tk p) (h d) -> p tk h d",
                                                p=P, h=HL)
                    for tk in range(TK):
                        nc.sync.dma_start(
                            out=vaug[:, tk, HL * r:HL * (r + 1), 0:DH],
                            in_=vg_r[:, tk, :, :])

                # Q^T from own query columns
                wq_r = wq_d.rearrange("(k p) m -> p k m", p=P)
                for m in range(CK):
                    wsl = pwstr.tile([P, CK, P], BF, tag="wsl", name=f"wq{m}")
                    nc.sync.dma_start(out=wsl, in_=wq_r[:, :, m * P:(m + 1) * P])
                    ps = psA.tile([P, 512], F32, tag="pa", name=f"psq{m}")
                    for k in range(CK):
                        nc.tensor.matmul(ps, wsl[:, k, :], xq_sb[:, k, :],
                                         start=(k == 0), stop=(k == CK - 1))
                    nc.vector.tensor_copy(qt_sb[:, m, :], ps)

            # -------- attention (head pairs; row-packed scores; fused exp) ----
            with (
                tc.tile_pool(name="pexp", bufs=2) as pexp,
                tc.tile_pool(name="pnrm", bufs=2) as pnrm,
                tc.tile_pool(name="psS", bufs=2, space="PSUM") as psS,
                tc.tile_pool(name="psPV", bufs=1, space="PSUM") as psPV,
                tc.tile_pool(name="psB", bufs=1, space="PSUM") as psB,
            ):
                for hc in range(CK):      # head pair = chunk hc: heads 2hc, 2hc+1
                    expT = pexp.tile([P, TK, 2, R], BF, tag="expT",
                                     name=f"expT{hc}")
                    for j in range(TK):
                        ps = psS.tile([P, 2, R], F32, tag="sc", name=f"sc{hc}_{j}")
                        nc.tensor.matmul(ps[:, 0, :],
                                         kt_sb[0:DH, hc, j * P:(j + 1) * P],
                                         qt_sb[0:DH, hc, :], start=True, stop=True)
                        nc.tensor.matmul(ps[:, 1, :],
                                         kt_sb[64:128, hc, j * P:(j + 1) * P],
                                         qt_sb[64:128, hc, :], start=True,
                                         stop=True)
                        nc.scalar.activation(expT[:, j, :, :], ps, AF.Exp,
                                             bias=zeros1, scale=0.125)
                    pvs = []
                    for par in range(2):
                        h = 2 * hc + par
                        pv = psPV.tile([P, R], F32, tag=f"pv{par}", name=f"pv{h}")
                        for j in range(TK):
                            nc.tensor.matmul(pv[0:DH + 1, :], vaug[:, j, h, :],
                                             expT[:, j, par, :],
                                             start=(j == 0), stop=(j == TK - 1))
                        pvs.append(pv)
                    for par in range(2):
                        h = 2 * hc + par
                        pv = pvs[par]
                        sums = pnrm.tile([P, R], F32, tag="sums", name=f"sm{h}")
                        nc.vector.tensor_copy(sums[0:1, :], pv[DH:DH + 1, :])
                        recip = pnrm.tile([P, R], F32, tag="recip", name=f"rc{h}")
                        nc.vector.reciprocal_approx_fast(recip[0:1, :],
                                                         sums[0:1, :])
                        pb = psB.tile([P, R], F32, tag="pb", name=f"pb{h}")
                        nc.tensor.matmul(pb[0:DH, :], ones_w[0:1, :],
                                         recip[0:1, :], start=True, stop=True)
                        bc_sb = pnrm.tile([P, R], F32, tag="bc", name=f"bc{h}")
                        nc.vector.tensor_copy(bc_sb[0:DH, :], pb[0:DH, :])
                        if par == 0:
                            nc.vector.tensor_tensor(out=ctxt[0:DH, hc, :],
                                                    in0=pv[0:DH, :],
                                                    in1=bc_sb[0:DH, :],
                                                    op=ALU.mult)
                        else:
                            ctmp = pnrm.tile([P, R], BF, tag="ctmp",
                                             name=f"ctmp{h}")
                            nc.vector.tensor_tensor(out=ctmp[0:DH, :],
                                                    in0=pv[0:DH, :],
                                                    in1=bc_sb[0:DH, :],
                                                    op=ALU.mult)
                            nc.vector.tensor_copy(ctxt[64:96, hc, :],
                                                  ctmp[0:32, :])
                            nc.vector.tensor_copy(ctxt[96:128, hc, :],
                                                  ctmp[32:64, :])

        # x1 (fp32 rows + transposed bf16) live from phase C into phase E
        mid = es.enter_context(tc.tile_pool(name="mid", bufs=1))
        x1 = mid.tile([P, RM, C], F32)
        x1T = mid.tile([P, CK, R], BF)

        # W2 + phase-E vectors: allocate early so the 8MB DMA overlaps C/D
        pw2 = es.enter_context(tc.tile_pool(name="pw2", bufs=1))
        w2_sb = pw2.tile([P, FK, C], BF, tag="w2")
        nc.sync.dma_start(out=w2_sb, in_=w2_d.rearrange("(k p) m -> p k m", p=P))
        if not trivial:
            g2b = pw2.tile([P, C], F32, tag="g2b")
            nc.sync.dma_start(out=g2b, in_=bcast(g2_d))
            be2b = pw2.tile([P, C], F32, tag="be2b")
            nc.sync.dma_start(out=be2b, in_=bcast(be2_d))
            b2b = pw2.tile([P, C], F32, tag="b2b")
            nc.sync.dma_start(out=b2b, in_=bcast(b2_d))

        # ============ Phase C: out-proj + residual1 + LN1 ============
        with (
            tc.tile_pool(name="pc", bufs=1) as pc,
            tc.tile_pool(name="psC", bufs=4, space="PSUM") as psC,
            tc.tile_pool(name="psT", bufs=2, space="PSUM") as psT,
            tc.tile_pool(name="ptmp", bufs=2) as ptmp,
        ):
            wo_sb = pc.tile([P, CK, C], BF, tag="wo")
            nc.sync.dma_start(out=wo_sb,
                              in_=wo_d.rearrange("(k p) m -> p k m", p=P))
            if not trivial:
                g1b = pc.tile([P, C], F32, tag="g1b")
                nc.sync.dma_start(out=g1b, in_=bcast(g1_d))
                be1b = pc.tile([P, C], F32, tag="be1b")
                nc.sync.dma_start(out=be1b, in_=bcast(be1_d))
            r1 = pc.tile([P, RM, C], F32, tag="r1")
            for m in range(RM):
                for n in range(2):
                    ps = psC.tile([P, 512], F32, tag="po", name=f"po{m}{n}")
                    for k in range(CK):
                        nc.tensor.matmul(ps, ctxt[:, k, m * P:(m + 1) * P],
                                         wo_sb[:, k, n * 512:(n + 1) * 512],
                                         start=(k == 0), stop=(k == CK - 1))
                    nc.vector.scalar_tensor_tensor(
                        out=r1[:, m, n * 512:(n + 1) * 512],
                        in0=x_rows[:, m, n * 512:(n + 1) * 512],
                        scalar=ALPHA, in1=ps, op0=ALU.mult, op1=ALU.add)
            for m in range(RM):
                stats = ptmp.tile([P, 2, 6], F32, tag="st", name=f"st{m}")
                for i in range(2):
                    nc.vector.bn_stats(out=stats[:, i, :],
                                       in_=r1[:, m, i * 512:(i + 1) * 512])
                mv = ptmp.tile([P, 2], F32, tag="mv", name=f"mv{m}")
                nc.vector.bn_aggr(out=mv, in_=stats)
                rstd = ptmp.tile([P, 1], F32, tag="rstd", name=f"rstd{m}")
                nc.scalar.activation(rstd, mv[:, 1:2], AF.Sqrt, bias=eps1)
                nc.vector.reciprocal(rstd, rstd)
                if trivial:
                    nc.vector.tensor_scalar(out=x1[:, m, :], in0=r1[:, m, :],
                                            scalar1=mv[:, 0:1], scalar2=rstd,
                                            op0=ALU.subtract, op1=ALU.mult)
                else:
                    tnorm = ptmp.tile([P, C], F32, tag="tn", name=f"tn{m}")
                    nc.vector.tensor_scalar(out=tnorm, in0=r1[:, m, :],
                                            scalar1=mv[:, 0:1], scalar2=rstd,
                                            op0=ALU.subtract, op1=ALU.mult)
                    nc.vector.tensor_tensor(out=tnorm, in0=tnorm, in1=g1b,
                                            op=ALU.mult)
                    nc.vector.tensor_tensor(out=x1[:, m, :], in0=tnorm, in1=be1b,
                                            op=ALU.add)
                x1bf = ptmp.tile([P, C], BF, tag="x1bf", name=f"x1bf{m}")
                nc.vector.tensor_copy(x1bf, x1[:, m, :])
                for c in range(CK):
                    pt = psT.tile([P, P], BF, tag="pt", name=f"pt{m}{c}")
                    nc.tensor.transpose(pt, x1bf[:, c * P:(c + 1) * P], ident)
                    nc.vector.tensor_copy(x1T[:, c, m * P:(m + 1) * P], pt)

        # ============ Phase D: FF1 + mish -> hT ============
        ffp = es.enter_context(tc.tile_pool(name="ffp", bufs=1))
        ht_sb = ffp.tile([P, FK, R], BF)

        with (
            tc.tile_pool(name="pw1", bufs=3) as pw1,
            tc.tile_pool(name="psF", bufs=3, space="PSUM") as psF,
            tc.tile_pool(name="mtmp", bufs=3) as mtmp,
        ):
            w1_r = w1_d.rearrange("(k p) m -> p k m", p=P)
            if trivial:
                # two f-chunks per iteration; fused [128, 2, 512] mish chain
                for m2 in range(FK // 2):
                    w1c = pw1.tile([P, CK, 2, P], BF, tag="w1c", name=f"w1c{m2}")
                    nc.sync.dma_start(
                        out=w1c,
                        in_=w1_r[:, :, 2 * m2 * P:(2 * m2 + 2) * P].rearrange(
                            "p k (a q) -> p k a q", a=2))
                    ps = psF.tile([P, 2, R], F32, tag="pf", name=f"pf{m2}")
                    for a in range(2):
                        for k in range(CK):
                            nc.tensor.matmul(ps[:, a, :], w1c[:, k, a, :],
                                             x1T[:, k, :],
                                             start=(k == 0), stop=(k == CK - 1))
                    # mish(z): w=exp(z); sq=(w+1)^2; u=1-2/(sq+1); h=z*u
                    w = mtmp.tile([P, 2, R], F32, tag="w", name=f"w{m2}")
                    nc.scalar.activation(w, ps, AF.Exp, bias=zeros1)
                    sq = mtmp.tile([P, 2, R], F32, tag="sq", name=f"sq{m2}")
                    nc.scalar.activation(sq, w, AF.Square, bias=ones1)
                    nc.vector.tensor_scalar_add(out=sq, in0=sq, scalar1=1.0)
                    rq = mtmp.tile([P, 2, R], F32, tag="rq", name=f"rq{m2}")
                    nc.vector.reciprocal_approx_fast(out=rq, in_=sq)
                    u = mtmp.tile([P, 2, R], F32, tag="u", name=f"u{m2}")
                    nc.scalar.activation(u, rq, AF.Identity, bias=ones1,
                                         scale=-2.0)
                    nc.vector.scalar_tensor_tensor(
                        out=ht_sb[:, 2 * m2:2 * m2 + 2, :], in0=ps,
                        scalar=0.0, in1=u, op0=ALU.add, op1=ALU.mult)
            else:
                for m in range(FK):
                    w1c = pw1.tile([P, CK, P], BF, tag="w1c", name=f"w1c{m}")
                    nc.sync.dma_start(out=w1c, in_=w1_r[:, :, m * P:(m + 1) * P])
                    ps = psF.tile([P, R], F32, tag="pf", name=f"pf{m}")
                    for k in range(CK):
                        nc.tensor.matmul(ps, w1c[:, k, :], x1T[:, k, :],
                                         start=(k == 0), stop=(k == CK - 1))
                    w = mtmp.tile([P, R], F32, tag="w", name=f"w{m}")
                    nc.scalar.activation(w, ps, AF.Exp, bias=b1t[:, m:m + 1])
                    sq = mtmp.tile([P, R], F32, tag="sq", name=f"sq{m}")
                    nc.scalar.activation(sq, w, AF.Square, bias=ones1)
                    nc.vector.tensor_scalar_add(out=sq, in0=sq, scalar1=1.0)
                    rq = mtmp.tile([P, R], F32, tag="rq", name=f"rq{m}")
                    nc.vector.reciprocal_approx_fast(out=rq, in_=sq)
                    u = mtmp.tile([P, R], F32, tag="u", name=f"u{m}")
                    nc.vector.scalar_tensor_tensor(out=u, in0=rq, scalar=-2.0,
                                                   in1=ones_f, op0=ALU.mult,
                                                   op1=ALU.add)
                    nc.vector.scalar_tensor_tensor(out=ht_sb[:, m, :], in0=ps,
                                                   scalar=b1t[:, m:m + 1], in1=u,
                                                   op0=ALU.add, op1=ALU.mult)

        # ============ Phase E: FF2 + residual2 + LN2 -> out ============
        with (
            tc.tile_pool(name="psO", bufs=2, space="PSUM") as psO,
            tc.tile_pool(name="otmp", bufs=2) as otmp,
        ):
            for m in range(RM):
                ps = psO.tile([P, 2, 512], F32, tag="po2", name=f"po2_{m}")
                for k in range(FK):
                    for n in range(2):
                        nc.tensor.matmul(ps[:, n, :],
                                         ht_sb[:, k, m * P:(m + 1) * P],
                                         w2_sb[:, k, n * 512:(n + 1) * 512],
                                         start=(k == 0), stop=(k == FK - 1))
                r2 = otmp.tile([P, C], F32, tag="r2", name=f"r2_{m}")
                for n in range(2):
                    nc.vector.scalar_tensor_tensor(
                        out=r2[:, n * 512:(n + 1) * 512],
                        in0=x1[:, m, n * 512:(n + 1) * 512],
                        scalar=ALPHA, in1=ps[:, n, :], op0=ALU.mult, op1=ALU.add)
                if not trivial:
                    nc.vector.tensor_tensor(out=r2, in0=r2, in1=b2b, op=ALU.add)
                stats = otmp.tile([P, 2, 6], F32, tag="st2", name=f"st2_{m}")
                for i in range(2):
                    nc.vector.bn_stats(out=stats[:, i, :],
                                       in_=r2[:, i * 512:(i + 1) * 512])
                mv = otmp.tile([P, 2], F32, tag="mv2", name=f"mv2_{m}")
                nc.vector.bn_aggr(out=mv, in_=stats)
                rstd = otmp.tile([P, 1], F32, tag="rstd2", name=f"rstd2_{m}")
                nc.scalar.activation(rstd, mv[:, 1:2], AF.Sqrt, bias=eps1)
                nc.vector.reciprocal(rstd, rstd)
                yout = otmp.tile([P, C], F32, tag="y", name=f"y{m}")
                nc.vector.tensor_scalar(out=yout, in0=r2,
                                        scalar1=mv[:, 0:1], scalar2=rstd,
                                        op0=ALU.subtract, op1=ALU.mult)
                if not trivial:
                    nc.vector.tensor_tensor(out=yout, in0=yout, in1=g2b,
                                            op=ALU.mult)
                    nc.vector.tensor_tensor(out=yout, in0=yout, in1=be2b,
                                            op=ALU.add)
                nc.sync.dma_start(
                    out=out_d.rearrange("(m p) c -> p m c", p=P)[:, m, :],
                    in_=yout)

    nc.compile()
    return nc


def _install_trace_hook():
    """Optional: enable NTFF profiling under axon (timing); best-effort."""
    import sys, types, ctypes
    so_path = "/opt/axon/libaxon_pjrt.so"
    try:
        lib = ctypes.CDLL(so_path)
        if not hasattr(lib, "axon_start_nrt_profile"):
            return False
        lib.axon_start_nrt_profile.argtypes = [ctypes.POINTER(ctypes.c_int64),
                                               ctypes.c_size_t]
        lib.axon_start_nrt_profile.restype = ctypes.c_int64
        lib.axon_stop_nrt_profile.argtypes = [ctypes.c_char_p]
        lib.axon_stop_nrt_profile.restype = ctypes.c_int64

        @contextlib.contextmanager
        def _hook(output_dir, device_ids):
            import jax
            jax.devices()
            if device_ids:
                ids = (ctypes.c_int64 * len(device_ids))(*device_ids)
                rc = lib.axon_start_nrt_profile(ids, len(device_ids))
            else:
                rc = lib.axon_start_nrt_profile(None, 0)
            if rc != 0:
                raise RuntimeError(f"axon_start_nrt_profile rc={rc}")
            try:
                yield
            finally:
                n = lib.axon_stop_nrt_profile(str(output_dir).encode())
                print(f"profile: {n} file(s) written to {output_dir}",
                      file=sys.stderr)

        mod = types.ModuleType("antenv.axon_hooks")
        mod.get_axon_ntff_profile_hook = lambda: _hook
        mod.set_axon_ntff_profile_hook = lambda h: None
        sys.modules["antenv.axon_hooks"] = mod
        import concourse.bass_utils as bu
        bu.upload_artifacts = lambda tmpdir: tmpdir
        return True
    except Exception:
        return False


def _make_in_maps(x, Wq, Wk, Wv, Wo, W1, b1, W2, b2, g1, be1, g2, be2):
    x = np.asarray(x, np.float32)
    bf = ml_dtypes.bfloat16
    Wk = np.asarray(Wk, np.float32)
    Wv = np.asarray(Wv, np.float32)
    shared = {
        "wq": np.asarray(Wq, np.float32).astype(bf),
        "wo": np.asarray(Wo, np.float32).astype(bf),
        "w1": np.asarray(W1, np.float32).astype(bf),
        "w2": np.asarray(W2, np.float32).astype(bf),
        "b1t": np.ascontiguousarray(np.asarray(b1, np.float32).reshape(FK, P).T),
        "b2": np.asarray(b2, np.float32), "g1": np.asarray(g1, np.float32),
        "be1": np.asarray(be1, np.float32), "g2": np.asarray(g2, np.float32),
        "be2": np.asarray(be2, np.float32),
    }
    in_maps = []
    for g in range(NCORES):
        b, r = g // 4, g % 4
        xT = np.ascontiguousarray(x[b].T.astype(bf))
        in_maps.append({
            "xT": xT,
            "xTq": np.ascontiguousarray(xT[:, R * r:R * (r + 1)]),
            "x_rows": np.ascontiguousarray(x[b, R * r:R * (r + 1), :]),
            "wk_sl": np.ascontiguousarray(Wk[:, CL * r:CL * (r + 1)]).astype(bf),
            "wv_sl": np.ascontiguousarray(Wv[:, CL * r:CL * (r + 1)]).astype(bf),
            **shared,
        })
    return in_maps


def kernel(x, Wq, Wk, Wv, Wo, W1, b1, W2, b2, g1, be1, g2, be2):
    in_maps = _make_in_maps(x, Wq, Wk, Wv, Wo, W1, b1, W2, b2, g1, be1, g2, be2)
    trivial = bool(
        np.allclose(np.asarray(g1), 1.0) and np.allclose(np.asarray(be1), 0.0)
        and np.allclose(np.asarray(g2), 1.0) and np.allclose(np.asarray(be2), 0.0)
        and np.allclose(np.asarray(b1), 0.0) and np.allclose(np.asarray(b2), 0.0))
    key = ("nc", trivial)
    if key not in _CACHE:
        _CACHE[key] = _build(trivial)
    nc = _CACHE[key]

    trace = os.environ.get("BASS_KERNEL_TRACE") == "1"
    kwargs = {}
    if trace and _install_trace_hook():
        kwargs = {"trace": True, "tmpdir": os.environ.get(
            "BASS_KERNEL_TRACE_DIR", "/tmp/kernel_trace")}
    res = run_bass_kernel_spmd(nc, in_maps, core_ids=list(range(NCORES)),
                               **kwargs)
    if trace and res.exec_time_ns is not None:
        print(f"HW exec time: {res.exec_time_ns} ns")
    out = np.empty((2, T, C), np.float32)
    for g in range(NCORES):
        b, r = g // 4, g % 4
        out[b, R * r:R * (r + 1), :] = res.results[g]["out"]
    return out


# revision 6
# speedup vs baseline: 1.1329x; 1.1329x over previous
"""DeepNorm encoder layer on 8 TRN2 NeuronCores (Bass/Tile, SPMD).

Sharding: batch (2) x row-chunks (4) data-parallel for queries/FF; K/V
projections tensor-parallel within each 4-core batch group (each core
projects 4 heads' K/V for the full sequence, AllGathered to all cores of
the group). Attention computes all 16 heads for the core's 512 query rows.

Matmuls run in bf16 with fp32 PSUM accumulation; residuals/LN in fp32.
Activations feeding matmul RHS are kept transposed ([feature, token]);
LN/residual rows are [token, feature]. Head pairs share one 2-bank PSUM
scores tile (row-packed K=64 matmuls) drained by a single fused ACT exp.
"""
import os
import contextlib
import numpy as np
import ml_dtypes

import concourse.bass as bass
import concourse.mybir as mybir
import concourse.tile as tile
from concourse import bacc
from concourse.bass_utils import run_bass_kernel_spmd
from concourse.masks import make_identity

P = 128
T = 2048          # sequence length
C = 1024          # d_model
H = 16            # heads
DH = 64           # head dim
FF = 4096         # d_ff
NCORES = 8
R = 512           # query rows per core
HL = H // 4       # heads projected locally per core (4)
CL = HL * DH      # local K/V width (256)
ALPHA = 2.0
EPS = 1e-5

BF = mybir.dt.bfloat16
F32 = mybir.dt.float32
AF = mybir.ActivationFunctionType
ALU = mybir.AluOpType

CK = C // P    # 8 chunks of d_model
FK = FF // P   # 32 chunks of d_ff
TK = T // P    # 16 chunks of sequence
NT = T // 512  # 4 free-dim tiles of sequence
RM = R // P    # 4 row chunks per core
GROUPS = [[0, 1, 2, 3], [4, 5, 6, 7]]

_CACHE = {}


def _build(trivial):
    """trivial=True: g1=g2=1, be1=be2=b1=b2=0 (the reference's actual
    parameters) -> skip the gain/bias application passes."""
    nc = bacc.Bacc("TRN2", target_bir_lowering=False, debug=False,
                   num_devices=NCORES)

    xT_d = nc.dram_tensor("xT", [C, T], BF, kind="ExternalInput").ap()
    xq_d = nc.dram_tensor("xTq", [C, R], BF, kind="ExternalInput").ap()
    xr_d = nc.dram_tensor("x_rows", [R, C], F32, kind="ExternalInput").ap()
    wq_d = nc.dram_tensor("wq", [C, C], BF, kind="ExternalInput").ap()
    wk_d = nc.dram_tensor("wk", [C, C], BF, kind="ExternalInput").ap()
    wv_d = nc.dram_tensor("wv", [C, C], BF, kind="ExternalInput").ap()
    wo_d = nc.dram_tensor("wo", [C, C], BF, kind="ExternalInput").ap()
    w1_d = nc.dram_tensor("w1", [C, FF], BF, kind="ExternalInput").ap()
    w2_d = nc.dram_tensor("w2", [FF, C], BF, kind="ExternalInput").ap()
    b1_d = nc.dram_tensor("b1t", [P, FK], F32, kind="ExternalInput").ap()
    b2_d = nc.dram_tensor("b2", [C], F32, kind="ExternalInput").ap()
    g1_d = nc.dram_tensor("g1", [C], F32, kind="ExternalInput").ap()
    be1_d = nc.dram_tensor("be1", [C], F32, kind="ExternalInput").ap()
    g2_d = nc.dram_tensor("g2", [C], F32, kind="ExternalInput").ap()
    be2_d = nc.dram_tensor("be2", [C], F32, kind="ExternalInput").ap()
    out_d = nc.dram_tensor("out", [R, C], F32, kind="ExternalOutput").ap()

    def bcast(ap):  # [N] dram vector -> AP broadcasting to 128 partitions
        return bass.AP(tensor=ap.tensor, offset=ap.offset,
                       ap=[[0, P]] + list(ap.ap))

    with tile.TileContext(nc) as tc, contextlib.ExitStack() as es:
        const = es.enter_context(tc.tile_pool(name="const", bufs=1))
        zeros1 = const.tile([P, 1], F32); nc.vector.memset(zeros1, 0.0)
        ones1 = const.tile([P, 1], F32); nc.vector.memset(ones1, 1.0)
        eps1 = const.tile([P, 1], F32); nc.vector.memset(eps1, EPS)
        ones_f = const.tile([P, 512], F32); nc.vector.memset(ones_f, 1.0)
        ones_w = const.tile([P, DH], F32); nc.vector.memset(ones_w, 1.0)
        ident = const.tile([P, P], BF); make_identity(nc, ident)
        b1t = const.tile([P, FK], F32); nc.sync.dma_start(out=b1t, in_=b1_d)

        rows_pool = es.enter_context(tc.tile_pool(name="rows", bufs=1))
        x_rows = rows_pool.tile([P, RM, C], F32)
        nc.sync.dma_start(out=x_rows, in_=xr_d.rearrange("(m p) c -> p m c", p=P))
        ctxt = rows_pool.tile([P, CK, R], BF)   # ctx^T [1024, 512]

        # ============ Phases A+B: projections + attention ============
        with tc.tile_pool(name="attn", bufs=1) as attn:
            qt_sb = attn.tile([P, CK, R], BF)           # Q^T [1024, 512]
            kt_sb = attn.tile([P, CK, T], BF)           # K^T [1024, 2048] (all heads)
            vaug = attn.tile([P, TK, H, DH + 1], BF)    # V rows + ones col

            with (
                tc.tile_pool(name="pa", bufs=1) as pa,
                tc.tile_pool(name="pwstr", bufs=4) as pwstr,
                tc.tile_pool(name="psA", bufs=4, space="PSUM") as psA,
            ):
                xt_sb = pa.tile([P, CK, T], BF, tag="xt")
                xt_r = xT_d.rearrange("(k p) t -> p k t", p=P)
                for k in range(CK):
                    for hh in range(2):
                        nc.sync.dma_start(
                            out=xt_sb[:, k, hh * 1024:(hh + 1) * 1024],
                            in_=xt_r[:, k, hh * 1024:(hh + 1) * 1024])
                xq_sb = pa.tile([P, CK, R], BF, tag="xq")
                nc.sync.dma_start(out=xq_sb,
                                  in_=xq_d.rearrange("(k p) t -> p k t", p=P))
                nc.vector.memset(vaug[:, :, :, DH:DH + 1], 1.0)

                # K^T full [C, T] computed locally
                wk_r = wk_d.rearrange("(k p) m -> p k m", p=P)
                for m in range(CK):
                    wsl = pwstr.tile([P, CK, P], BF, tag="wsl", name=f"wk{m}")
                    nc.sync.dma_start(out=wsl, in_=wk_r[:, :, m * P:(m + 1) * P])
                    for n in range(NT):
                        ps = psA.tile([P, 512], F32, tag="pa", name=f"psk{m}{n}")
                        for k in range(CK):
                            nc.tensor.matmul(ps, wsl[:, k, :],
                                             xt_sb[:, k, n * 512:(n + 1) * 512],
                                             start=(k == 0), stop=(k == CK - 1))
                        nc.vector.tensor_copy(kt_sb[:, m, n * 512:(n + 1) * 512],
                                              ps)
                # V full [T, C] locally, evicted into vaug strided per head
                wv_r = wv_d.rearrange("(k p) m -> p k m", p=P)
                for n in range(2):
                    wsl = pwstr.tile([P, CK, 512], BF, tag="wsl2", bufs=2,
                                     name=f"wv{n}")
                    nc.sync.dma_start(out=wsl,
                                      in_=wv_r[:, :, n * 512:(n + 1) * 512])
                    for t in range(TK):
                        ps = psA.tile([P, 512], F32, tag="pa", name=f"psv{t}{n}")
                        for k in range(CK):
                            nc.tensor.matmul(ps, xt_sb[:, k, t * P:(t + 1) * P],
                                             wsl[:, k, :],
                                             start=(k == 0), stop=(k == CK - 1))
                        nc.vector.tensor_copy(
                            vaug[:, t, n * 8:(n + 1) * 8, 0:DH],
                            ps.rearrange("p (h d) -> p h d", h=8))

                # Q^T from own query columns
                wq_r = wq_d.rearrange("(k p) m -> p k m", p=P)
                for m in range(CK):
                    wsl = pwstr.tile([P, CK, P], BF, tag="wsl", name=f"wq{m}")
                    nc.sync.dma_start(out=wsl, in_=wq_r[:, :, m * P:(m + 1) * P])
                    ps = psA.tile([P, 512], F32, tag="pa", name=f"psq{m}")
                    for k in range(CK):
                        nc.tensor.matmul(ps, wsl[:, k, :], xq_sb[:, k, :],
                                         start=(k == 0), stop=(k == CK - 1))
                    nc.vector.tensor_copy(qt_sb[:, m, :], ps)

            # -------- attention (head pairs; row-packed scores; fused exp) ----
            with (
                tc.tile_pool(name="pexp", bufs=2) as pexp,
                tc.tile_pool(name="pnrm", bufs=2) as pnrm,
                tc.tile_pool(name="psS", bufs=2, space="PSUM") as psS,
                tc.tile_pool(name="psPV", bufs=1, space="PSUM") as psPV,
                tc.tile_pool(name="psB", bufs=1, space="PSUM") as psB,
            ):
                for hc in range(CK):      # head pair = chunk hc: heads 2hc, 2hc+1
                    expT = pexp.tile([P, TK, 2, R], BF, tag="expT",
                                     name=f"expT{hc}")
                    for j in range(TK):
                        ps = psS.tile([P, 2, R], F32, tag="sc", name=f"sc{hc}_{j}")
                        nc.tensor.matmul(ps[:, 0, :],
                                         kt_sb[0:DH, hc, j * P:(j + 1) * P],
                                         qt_sb[0:DH, hc, :], start=True, stop=True)
                        nc.tensor.matmul(ps[:, 1, :],
                                         kt_sb[64:128, hc, j * P:(j + 1) * P],
                                         qt_sb[64:128, hc, :], start=True,
                                         stop=True)
                        nc.scalar.activation(expT[:, j, :, :], ps, AF.Exp,
                                             bias=zeros1, scale=0.125)
                    pvs = []
                    for par in range(2):
                        h = 2 * hc + par
                        pv = psPV.tile([P, R], F32, tag=f"pv{par}", name=f"pv{h}")
                        for j in range(TK):
                            nc.tensor.matmul(pv[0:DH + 1, :], vaug[:, j, h, :],
                                             expT[:, j, par, :],
                                             start=(j == 0), stop=(j == TK - 1))
                        pvs.append(pv)
                    for par in range(2):
                        h = 2 * hc + par
                        pv = pvs[par]
                        sums = pnrm.tile([P, R], F32, tag="sums", name=f"sm{h}")
                        nc.vector.tensor_copy(sums[0:1, :], pv[DH:DH + 1, :])
                        recip = pnrm.tile([P, R], F32, tag="recip", name=f"rc{h}")
                        nc.vector.reciprocal_approx_fast(recip[0:1, :],
                                                         sums[0:1, :])
                        pb = psB.tile([P, R], F32, tag="pb", name=f"pb{h}")
                        nc.tensor.matmul(pb[0:DH, :], ones_w[0:1, :],
                                         recip[0:1, :], start=True, stop=True)
                        bc_sb = pnrm.tile([P, R], F32, tag="bc", name=f"bc{h}")
                        nc.vector.tensor_copy(bc_sb[0:DH, :], pb[0:DH, :])
                        if par == 0:
                            nc.vector.tensor_tensor(out=ctxt[0:DH, hc, :],
                                                    in0=pv[0:DH, :],
                                                    in1=bc_sb[0:DH, :],
                                                    op=ALU.mult)
                        else:
                            ctmp = pnrm.tile([P, R], BF, tag="ctmp",
                                             name=f"ctmp{h}")
                            nc.vector.tensor_tensor(out=ctmp[0:DH, :],
                                                    in0=pv[0:DH, :],
                                                    in1=bc_sb[0:DH, :],
                                                    op=ALU.mult)
                            nc.vector.tensor_copy(ctxt[64:96, hc, :],
                                                  ctmp[0:32, :])
                            nc.vector.tensor_copy(ctxt[96:128, hc, :],
                                                  ctmp[32:64, :])

        # x1 (fp32 rows + transposed bf16) live from phase C into phase E
        mid = es.enter_context(tc.tile_pool(name="mid", bufs=1))
        x1 = mid.tile([P, RM, C], F32)
        x1T = mid.tile([P, CK, R], BF)

        # W2 + phase-E vectors: allocate early so the 8MB DMA overlaps C/D
        pw2 = es.enter_context(tc.tile_pool(name="pw2", bufs=1))
        w2_sb = pw2.tile([P, FK, C], BF, tag="w2")
        nc.sync.dma_start(out=w2_sb, in_=w2_d.rearrange("(k p) m -> p k m", p=P))
        if not trivial:
            g2b = pw2.tile([P, C], F32, tag="g2b")
            nc.sync.dma_start(out=g2b, in_=bcast(g2_d))
            be2b = pw2.tile([P, C], F32, tag="be2b")
            nc.sync.dma_start(out=be2b, in_=bcast(be2_d))
            b2b = pw2.tile([P, C], F32, tag="b2b")
            nc.sync.dma_start(out=b2b, in_=bcast(b2_d))

        # ============ Phase C: out-proj + residual1 + LN1 ============
        with (
            tc.tile_pool(name="pc", bufs=1) as pc,
            tc.tile_pool(name="psC", bufs=4, space="PSUM") as psC,
            tc.tile_pool(name="psT", bufs=2, space="PSUM") as psT,
            tc.tile_pool(name="ptmp", bufs=2) as ptmp,
        ):
            wo_sb = pc.tile([P, CK, C], BF, tag="wo")
            nc.sync.dma_start(out=wo_sb,
                              in_=wo_d.rearrange("(k p) m -> p k m", p=P))
            if not trivial:
                g1b = pc.tile([P, C], F32, tag="g1b")
                nc.sync.dma_start(out=g1b, in_=bcast(g1_d))
                be1b = pc.tile([P, C], F32, tag="be1b")
                nc.sync.dma_start(out=be1b, in_=bcast(be1_d))
            r1 = pc.tile([P, RM, C], F32, tag="r1")
            for m in range(RM):
                for n in range(2):
                    ps = psC.tile([P, 512], F32, tag="po", name=f"po{m}{n}")
                    for k in range(CK):
                        nc.tensor.matmul(ps, ctxt[:, k, m * P:(m + 1) * P],
                                         wo_sb[:, k, n * 512:(n + 1) * 512],
                                         start=(k == 0), stop=(k == CK - 1))
                    nc.vector.scalar_tensor_tensor(
                        out=r1[:, m, n * 512:(n + 1) * 512],
                        in0=x_rows[:, m, n * 512:(n + 1) * 512],
                        scalar=ALPHA, in1=ps, op0=ALU.mult, op1=ALU.add)
            for m in range(RM):
                stats = ptmp.tile([P, 2, 6], F32, tag="st", name=f"st{m}")
                for i in range(2):
                    nc.vector.bn_stats(out=stats[:, i, :],
                                       in_=r1[:, m, i * 512:(i + 1) * 512])
                mv = ptmp.tile([P, 2], F32, tag="mv", name=f"mv{m}")
                nc.vector.bn_aggr(out=mv, in_=stats)
                rstd = ptmp.tile([P, 1], F32, tag="rstd", name=f"rstd{m}")
                nc.scalar.activation(rstd, mv[:, 1:2], AF.Sqrt, bias=eps1)
                nc.vector.reciprocal(rstd, rstd)
                if trivial:
                    nc.vector.tensor_scalar(out=x1[:, m, :], in0=r1[:, m, :],
                                            scalar1=mv[:, 0:1], scalar2=rstd,
                                            op0=ALU.subtract, op1=ALU.mult)
                else:
                    tnorm = ptmp.tile([P, C], F32, tag="tn", name=f"tn{m}")
                    nc.vector.tensor_scalar(out=tnorm, in0=r1[:, m, :],
                                            scalar1=mv[:, 0:1], scalar2=rstd,
                                            op0=ALU.subtract, op1=ALU.mult)
                    nc.vector.tensor_tensor(out=tnorm, in0=tnorm, in1=g1b,
                                            op=ALU.mult)
                    nc.vector.tensor_tensor(out=x1[:, m, :], in0=tnorm, in1=be1b,
                                            op=ALU.add)
                x1bf = ptmp.tile([P, C], BF, tag="x1bf", name=f"x1bf{m}")
                nc.scalar.copy(x1bf, x1[:, m, :])
                for c in range(CK):
                    pt = psT.tile([P, P], BF, tag="pt", name=f"pt{m}{c}")
                    nc.tensor.transpose(pt, x1bf[:, c * P:(c + 1) * P], ident)
                    nc.scalar.copy(x1T[:, c, m * P:(m + 1) * P], pt)

        # ============ Phase D: FF1 + mish -> hT ============
        ffp = es.enter_context(tc.tile_pool(name="ffp", bufs=1))
        ht_sb = ffp.tile([P, FK, R], BF)

        with (
            tc.tile_pool(name="pw1", bufs=3) as pw1,
            tc.tile_pool(name="psF", bufs=3, space="PSUM") as psF,
            tc.tile_pool(name="mtmp", bufs=3) as mtmp,
        ):
            w1_r = w1_d.rearrange("(k p) m -> p k m", p=P)
            if trivial:
                # two f-chunks per iteration; fused [128, 2, 512] mish chain
                for m2 in range(FK // 2):
                    w1c = pw1.tile([P, CK, 2, P], BF, tag="w1c", name=f"w1c{m2}")
                    nc.sync.dma_start(
                        out=w1c,
                        in_=w1_r[:, :, 2 * m2 * P:(2 * m2 + 2) * P].rearrange(
                            "p k (a q) -> p k a q", a=2))
                    ps = psF.tile([P, 2, R], F32, tag="pf", name=f"pf{m2}")
                    for a in range(2):
                        for k in range(CK):
                            nc.tensor.matmul(ps[:, a, :], w1c[:, k, a, :],
                                             x1T[:, k, :],
                                             start=(k == 0), stop=(k == CK - 1))
                    # mish(z): w=exp(z); sq=(w+1)^2; u=1-2/(sq+1); h=z*u
                    w = mtmp.tile([P, 2, R], F32, tag="w", name=f"w{m2}")
                    nc.scalar.activation(w, ps, AF.Exp, bias=zeros1)
                    sq = mtmp.tile([P, 2, R], F32, tag="sq", name=f"sq{m2}")
                    nc.scalar.activation(sq, w, AF.Square, bias=ones1)
                    nc.vector.tensor_scalar_add(out=sq, in0=sq, scalar1=1.0)
                    rq = mtmp.tile([P, 2, R], F32, tag="rq", name=f"rq{m2}")
                    nc.vector.reciprocal_approx_fast(out=rq, in_=sq)
                    u = mtmp.tile([P, 2, R], F32, tag="u", name=f"u{m2}")
                    nc.scalar.activation(u, rq, AF.Identity, bias=ones1,
                                         scale=-2.0)
                    nc.vector.scalar_tensor_tensor(
                        out=ht_sb[:, 2 * m2:2 * m2 + 2, :], in0=ps,
                        scalar=0.0, in1=u, op0=ALU.add, op1=ALU.mult)
            else:
                for m in range(FK):
                    w1c = pw1.tile([P, CK, P], BF, tag="w1c", name=f"w1c{m}")
                    nc.sync.dma_start(out=w1c, in_=w1_r[:, :, m * P:(m + 1) * P])
                    ps = psF.tile([P, R], F32, tag="pf", name=f"pf{m}")
                    for k in range(CK):
                        nc.tensor.matmul(ps, w1c[:, k, :], x1T[:, k, :],
                                         start=(k == 0), stop=(k == CK - 1))
                    w = mtmp.tile([P, R], F32, tag="w", name=f"w{m}")
                    nc.scalar.activation(w, ps, AF.Exp, bias=b1t[:, m:m + 1])
                    sq = mtmp.tile([P, R], F32, tag="sq", name=f"sq{m}")
                    nc.scalar.activation(sq, w, AF.Square, bias=ones1)
                    nc.vector.tensor_scalar_add(out=sq, in0=sq, scalar1=1.0)
                    rq = mtmp.tile([P, R], F32, tag="rq", name=f"rq{m}")
                    nc.vector.reciprocal_approx_fast(out=rq, in_=sq)
                    u = mtmp.tile([P, R], F32, tag="u", name=f"u{m}")
                    nc.vector.scalar_tensor_tensor(out=u, in0=rq, scalar=-2.0,
                                                   in1=ones_f, op0=ALU.mult,
                                                   op1=ALU.add)
                    nc.vector.scalar_tensor_tensor(out=ht_sb[:, m, :], in0=ps,
                                                   scalar=b1t[:, m:m + 1], in1=u,
                                                   op0=ALU.add, op1=ALU.mult)

        # ============ Phase E: FF2 + residual2 + LN2 -> out ============
        with (
            tc.tile_pool(name="psO", bufs=2, space="PSUM") as psO,
            tc.tile_pool(name="otmp", bufs=2) as otmp,
        ):
            for m in range(RM):
                ps = psO.tile([P, 2, 512], F32, tag="po2", name=f"po2_{m}")
                for k in range(FK):
                    for n in range(2):
                        nc.tensor.matmul(ps[:, n, :],
                                         ht_sb[:, k, m * P:(m + 1) * P],
                                         w2_sb[:, k, n * 512:(n + 1) * 512],
                                         start=(k == 0), stop=(k == FK - 1))
                r2 = otmp.tile([P, C], F32, tag="r2", name=f"r2_{m}")
                for n in range(2):
                    nc.vector.scalar_tensor_tensor(
                        out=r2[:, n * 512:(n + 1) * 512],
                        in0=x1[:, m, n * 512:(n + 1) * 512],
                        scalar=ALPHA, in1=ps[:, n, :], op0=ALU.mult, op1=ALU.add)
                if not trivial:
                    nc.vector.tensor_tensor(out=r2, in0=r2, in1=b2b, op=ALU.add)
                stats = otmp.tile([P, 2, 6], F32, tag="st2", name=f"st2_{m}")
                for i in range(2):
                    nc.vector.bn_stats(out=stats[:, i, :],
                                       in_=r2[:, i * 512:(i + 1) * 512])
                mv = otmp.tile([P, 2], F32, tag="mv2", name=f"mv2_{m}")
                nc.vector.bn_aggr(out=mv, in_=stats)
                rstd = otmp.tile([P, 1], F32, tag="rstd2", name=f"rstd2_{m}")
                nc.scalar.activation(rstd, mv[:, 1:2], AF.Sqrt, bias=eps1)
                nc.vector.reciprocal(rstd, rstd)
                yout = otmp.tile([P, C], F32, tag="y", name=f"y{m}")
                nc.vector.tensor_scalar(out=yout, in0=r2,
                                        scalar1=mv[:, 0:1], scalar2=rstd,
                                        op0=ALU.subtract, op1=ALU.mult)
                if not trivial:
                    nc.vector.tensor_tensor(out=yout, in0=yout, in1=g2b,
                                            op=ALU.mult)
                    nc.vector.tensor_tensor(out=yout, in0=yout, in1=be2b,
                                            op=ALU.add)
                nc.sync.dma_start(
                    out=out_d.rearrange("(m p) c -> p m c", p=P)[:, m, :],
                    in_=yout)

    nc.compile()
    return nc


def _install_trace_hook():
    """Optional: enable NTFF profiling under axon (timing); best-effort."""
    import sys, types, ctypes
    so_path = "/opt/axon/libaxon_pjrt.so"
    try:
        lib = ctypes.CDLL(so_path)
        if not hasattr(lib, "axon_start_nrt_profile"):
            return False
        lib.axon_start_nrt_profile.argtypes = [ctypes.POINTER(ctypes.c_int64),
                                               ctypes.c_size_t]
        lib.axon_start_nrt_profile.restype = ctypes.c_int64
        lib.axon_stop_nrt_profile.argtypes = [ctypes.c_char_p]
        lib.axon_stop_nrt_profile.restype = ctypes.c_int64

        @contextlib.contextmanager
        def _hook(output_dir, device_ids):
            import jax
            jax.devices()
            if device_ids:
                ids = (ctypes.c_int64 * len(device_ids))(*device_ids)
                rc = lib.axon_start_nrt_profile(ids, len(device_ids))
            else:
                rc = lib.axon_start_nrt_profile(None, 0)
            if rc != 0:
                raise RuntimeError(f"axon_start_nrt_profile rc={rc}")
            try:
                yield
            finally:
                n = lib.axon_stop_nrt_profile(str(output_dir).encode())
                print(f"profile: {n} file(s) written to {output_dir}",
                      file=sys.stderr)

        mod = types.ModuleType("antenv.axon_hooks")
        mod.get_axon_ntff_profile_hook = lambda: _hook
        mod.set_axon_ntff_profile_hook = lambda h: None
        sys.modules["antenv.axon_hooks"] = mod
        import concourse.bass_utils as bu
        bu.upload_artifacts = lambda tmpdir: tmpdir
        return True
    except Exception:
        return False


def _make_in_maps(x, Wq, Wk, Wv, Wo, W1, b1, W2, b2, g1, be1, g2, be2):
    x = np.asarray(x, np.float32)
    bf = ml_dtypes.bfloat16
    shared = {
        "wq": np.asarray(Wq, np.float32).astype(bf),
        "wk": np.asarray(Wk, np.float32).astype(bf),
        "wv": np.asarray(Wv, np.float32).astype(bf),
        "wo": np.asarray(Wo, np.float32).astype(bf),
        "w1": np.asarray(W1, np.float32).astype(bf),
        "w2": np.asarray(W2, np.float32).astype(bf),
        "b1t": np.ascontiguousarray(np.asarray(b1, np.float32).reshape(FK, P).T),
        "b2": np.asarray(b2, np.float32), "g1": np.asarray(g1, np.float32),
        "be1": np.asarray(be1, np.float32), "g2": np.asarray(g2, np.float32),
        "be2": np.asarray(be2, np.float32),
    }
    in_maps = []
    for g in range(NCORES):
        b, r = g // 4, g % 4
        xT = np.ascontiguousarray(x[b].T.astype(bf))
        in_maps.append({
            "xT": xT,
            "xTq": np.ascontiguousarray(xT[:, R * r:R * (r + 1)]),
            "x_rows": np.ascontiguousarray(x[b, R * r:R * (r + 1), :]),
            **shared,
        })
    return in_maps


def kernel(x, Wq, Wk, Wv, Wo, W1, b1, W2, b2, g1, be1, g2, be2):
    in_maps = _make_in_maps(x, Wq, Wk, Wv, Wo, W1, b1, W2, b2, g1, be1, g2, be2)
    trivial = bool(
        np.allclose(np.asarray(g1), 1.0) and np.allclose(np.asarray(be1), 0.0)
        and np.allclose(np.asarray(g2), 1.0) and np.allclose(np.asarray(be2), 0.0)
        and np.allclose(np.asarray(b1), 0.0) and np.allclose(np.asarray(b2), 0.0))
    key = ("nc", trivial)
    if key not in _CACHE:
        _CACHE[key] = _build(trivial)
    nc = _CACHE[key]

    trace = os.environ.get("BASS_KERNEL_TRACE") == "1"
    kwargs = {}
    if trace and _install_trace_hook():
        kwargs = {"trace": True, "tmpdir": os.environ.get(
            "BASS_KERNEL_TRACE_DIR", "/tmp/kernel_trace")}
    res = run_bass_kernel_spmd(nc, in_maps, core_ids=list(range(NCORES)),
                               **kwargs)
    if trace and res.exec_time_ns is not None:
        print(f"HW exec time: {res.exec_time_ns} ns")
    out = np.empty((2, T, C), np.float32)
    for g in range(NCORES):
        b, r = g // 4, g % 4
        out[b, R * r:R * (r + 1), :] = res.results[g]["out"]
    return out
